# revision 11
# baseline (speedup 1.0000x reference)
"""Chamfer distance kernel for Trainium2 (8 NeuronCores, SPMD).

Strategy
--------
x is sharded across 8 cores (2048 rows each); y (16384 points) is replicated.
Each core computes its [2048, 16384] block of the squared-distance matrix
d_ij = |x_i - y_j|^2 via K=13 bf16 matmuls and reduces it on the fly:

 * row mins (dist1 shard) via a fused tensor_tensor_reduce per PSUM group
 * column mins (dist2 partial, min over the core's 2048 rows) via fp16
   tensor_tensor min into a resident [128, 16384] buffer, folded across
   partitions at the end with PE transposes + reduces.

Numerical trick: y is kd-sorted into 32 spatially compact tiles of 512; both
point sets are translated by the tile centroid before augmentation
(d is translation invariant), and each translated coordinate is split into
bf16 hi+lo parts. All PE products are then exact and small, so the K=13
bf16 matmul reproduces d to ~f32 quality despite the catastrophic
cancellation in x2+y2-2xy (measured end-to-end rel err ~8e-5).

The host finishes with a trivial O(N) reduction: sum of row mins, min of the
8 per-core column-min vectors, mean.
"""
import sys

sys.path.insert(0, "/opt/trn_rl_repo")

import numpy as np
import ml_dtypes

import concourse.bass as bass
import concourse.tile as tile
from concourse import bacc, mybir
from concourse import bass_utils
from concourse.bass_isa import ReduceOp

BF16 = ml_dtypes.bfloat16

# Problem geometry (hardcoded per the task contract).
N = 16384          # x points
M = 16384          # y points
D = 3
NCORES = 8
XSHARD = N // NCORES        # 2048 x rows per core
P = 128                     # partitions
YTILE = 512                 # translation granularity == matmul moving width
NYT = M // YTILE            # 32 y tiles
YGRP = 2048                 # PSUM group width (4 banks)
NYG = M // YGRP             # 8 groups
NXT = XSHARD // P           # 16 x tiles per core
K = 13                      # augmented contraction depth
VPER = 3                    # XT variants packed per 128-partition page
NPAGES = (NYT + VPER - 1) // VPER   # 11
NCHUNK = M // P             # 128 column chunks of C
INF = 3.0e38
GP_PARITY = 1   # yg slices with this parity run their C-update on GPSIMD


def _bf16_pair(a):
    """Split float64 array into (hi, lo) bf16 parts."""
    hi = a.astype(BF16)
    lo = (a - hi.astype(np.float64)).astype(BF16)
    return hi, lo


def kd_sort(y, n_tiles):
    """Recursive median splits -> permutation grouping y into n_tiles
    spatially compact tiles (n_tiles must be a power of two)."""
    groups = [np.arange(len(y))]
    while len(groups) < n_tiles:
        nxt = []
        for g in groups:
            pts = y[g]
            dim = int(np.argmax(pts.max(0) - pts.min(0)))
            order = np.argsort(pts[:, dim], kind="stable")
            half = len(g) // 2
            nxt.append(g[order[:half]])
            nxt.append(g[order[half:]])
        groups = nxt
    return np.concatenate(groups)


def build_nc(n_xt=NXT, n_yg=NYG):
    """Build the SPMD Bass program (same NEFF on all cores)."""
    n_yt = n_yg * (YGRP // YTILE)
    n_pages = (n_yt + VPER - 1) // VPER
    m = n_yt * YTILE
    n_chunk = m // P
    xshard = n_xt * P
    xt_cols = n_pages * xshard
    out_w = n_xt

    nc = bacc.Bacc("TRN2", target_bir_lowering=False, debug=False,
                   num_devices=NCORES)
    xt_d = nc.dram_tensor("xt", [P, xt_cols], mybir.dt.bfloat16,
                          kind="ExternalInput")
    yt_d = nc.dram_tensor("yt", [64 + K, m], mybir.dt.bfloat16,
                          kind="ExternalInput")
    out_d = nc.dram_tensor("out", [P, out_w], mybir.dt.float32,
                           kind="ExternalOutput")
    d2_d = nc.dram_tensor("d2", [1, m], mybir.dt.float16,
                          kind="ExternalOutput")

    with tile.TileContext(nc) as tc:
        with (
            tc.tile_pool(name="const", bufs=1) as cpool,
            tc.tile_pool(name="spool", bufs=3) as spool,
            tc.tile_pool(name="apool", bufs=2) as apool,
            tc.tile_pool(name="ps", bufs=2, space="PSUM") as pspool,
        ):
            xt_t = cpool.tile([P, xt_cols], mybir.dt.bfloat16)
            yt_t = cpool.tile([64 + K, m], mybir.dt.bfloat16)
            c_t = cpool.tile([P, m], mybir.dt.float16)
            out_t = cpool.tile([P, out_w], mybir.dt.float32)
            nc.sync.dma_start(xt_t[:], xt_d.ap())
            nc.sync.dma_start(yt_t[:], yt_d.ap())

            for xt in range(n_xt):
                a_t = apool.tile([P, n_yg], mybir.dt.float32, tag="acc")
                for yg in range(n_yg):
                    ps = pspool.tile([P, YGRP], mybir.dt.float32, tag="d")
                    for j4 in range(YGRP // YTILE):
                        j = yg * (YGRP // YTILE) + j4
                        page, slot = divmod(j, VPER)
                        lhsT = xt_t[slot * 32: slot * 32 + K,
                                    page * xshard + xt * P:
                                    page * xshard + (xt + 1) * P]
                        nc.tensor.matmul(
                            ps[:, j4 * YTILE:(j4 + 1) * YTILE],
                            lhsT,
                            yt_t[slot * 32: slot * 32 + K,
                                 j * YTILE:(j + 1) * YTILE],
                            start=True, stop=True,
                        )
                    s_t = spool.tile([P, YGRP], mybir.dt.float16, tag="s")
                    cs = c_t[:, yg * YGRP:(yg + 1) * YGRP]
                    # A[:, yg] = min_j d (clean row mins); S = fp16 copy of d
                    nc.vector.tensor_reduce(
                        a_t[:, yg:yg + 1], ps[:],
                        axis=mybir.AxisListType.X, op=mybir.AluOpType.min,
                    )
                    nc.scalar.mul(s_t[:], ps[:], -1.0)
                    if xt == 0:
                        nc.vector.tensor_copy(cs, s_t[:])
                    else:
                        nc.vector.tensor_tensor(cs, s_t[:], cs,
                                                mybir.AluOpType.max)
                nc.vector.tensor_reduce(
                    out_t[:, xt:xt + 1], a_t[:],
                    axis=mybir.AxisListType.X, op=mybir.AluOpType.min,
                )

            # Fold C (holding -d maxes) across partitions in place on GPSIMD.
            nc.gpsimd.partition_all_reduce(c_t[:], c_t[:], P, ReduceOp.max)
            nc.sync.dma_start(d2_d.ap(), c_t[0:1, :])
            nc.sync.dma_start(out_d.ap(), out_t[:])

    nc.compile()
    return nc


def prep_inputs(x, y, n_xt=NXT, n_yg=NYG):
    """Host-side: kd-sort y, per-tile translate+augment+bf16-split, pack."""
    n_yt = n_yg * (YGRP // YTILE)
    n_pages = (n_yt + VPER - 1) // VPER
    m = n_yt * YTILE
    xshard = n_xt * P
    ncores = x.shape[0] // xshard

    perm = kd_sort(y, n_yt)
    ys = y[perm].astype(np.float64)

    yt = np.zeros((K, m), dtype=BF16)
    xts = [np.zeros((P, n_pages * xshard), dtype=BF16) for _ in range(ncores)]
    x64 = x.astype(np.float64)

    for j in range(n_yt):
        sl = slice(j * YTILE, (j + 1) * YTILE)
        yb = ys[sl]
        c = yb.mean(0)
        yp = yb - c
        yh, yl = _bf16_pair(yp)
        y2h, y2l = _bf16_pair((yp ** 2).sum(1))
        # rhs rows: yh(3), yl(3), yh(3), 1, 1, y2h, y2l
        yt[0:3, sl] = yh.T
        yt[3:6, sl] = yl.T
        yt[6:9, sl] = yh.T
        yt[9, sl] = BF16(1.0)
        yt[10, sl] = BF16(1.0)
        yt[11, sl] = y2h
        yt[12, sl] = y2l

        page, slot = divmod(j, VPER)
        xp_all = x64 - c
        x2_all = (xp_all ** 2).sum(1)
        for cidx in range(ncores):
            xp = xp_all[cidx * xshard:(cidx + 1) * xshard]
            x2 = x2_all[cidx * xshard:(cidx + 1) * xshard]
            xh, xl = _bf16_pair(xp)
            m2h = (-2.0 * xh.astype(np.float64)).astype(BF16)
            m2l = (-2.0 * xl.astype(np.float64)).astype(BF16)
            x2h, x2l = _bf16_pair(x2)
            blk = np.zeros((K, xshard), dtype=BF16)
            # lhsT rows paired with rhs rows above:
            blk[0:3] = m2h.T          # . yh
            blk[3:6] = m2h.T          # . yl
            blk[6:9] = m2l.T          # . yh
            blk[9] = x2h              # . 1
            blk[10] = x2l             # . 1
            blk[11] = BF16(1.0)       # . y2h
            blk[12] = BF16(1.0)       # . y2l
            xts[cidx][slot * 32: slot * 32 + K,
                      page * xshard:(page + 1) * xshard] = blk

    yt_rep = np.zeros((64 + K, m), dtype=BF16)
    for s in range(VPER):
        yt_rep[s * 32: s * 32 + K] = yt
    in_maps = [
        {"xt": xts[cidx], "yt": yt_rep}
        for cidx in range(ncores)
    ]
    return in_maps


def postprocess(results, n_xt=NXT, n_yg=NYG):
    n_yt = n_yg * (YGRP // YTILE)
    m = n_yt * YTILE
    d1_sum = 0.0
    d2 = np.full(m, np.inf, np.float64)
    for res in results:
        out = res["out"].astype(np.float64)
        d1_sum += np.maximum(out[:, :n_xt], 0.0).sum()
        d2 = np.minimum(d2, -res["d2"][0].astype(np.float64))
    d2_sum = np.maximum(d2, 0.0).sum()
    n_x = n_xt * P * len(results)
    return (d1_sum + d2_sum) / (n_x + m)


_NC_CACHE = {}


def kernel(x, y):
    x = np.asarray(x, np.float32)
    y = np.asarray(y, np.float32)
    key = "full"
    if key not in _NC_CACHE:
        _NC_CACHE[key] = build_nc()
    nc = _NC_CACHE[key]
    in_maps = prep_inputs(x, y)
    res = bass_utils.run_bass_kernel_spmd(nc, in_maps,
                                          core_ids=list(range(NCORES)))
    val = postprocess(res.results)
    return np.array(val, dtype=np.float32)


if __name__ == "__main__":
    np.random.seed(0)
    x = np.random.randn(N, D).astype(np.float32)
    y = np.random.randn(M, D).astype(np.float32)
    print("kernel:", kernel(x, y))


# revision 12
# speedup vs baseline: 1.0391x; 1.0391x over previous
"""Chamfer distance kernel for Trainium2 (8 NeuronCores, SPMD).

Strategy
--------
x is sharded across 8 cores (2048 rows each); y (16384 points) is replicated.
Each core computes its [2048, 16384] block of the squared-distance matrix
d_ij = |x_i - y_j|^2 via K=13 bf16 matmuls and reduces it on the fly:

 * row mins (dist1 shard) via a fused tensor_tensor_reduce per PSUM group
 * column mins (dist2 partial, min over the core's 2048 rows) via fp16
   tensor_tensor min into a resident [128, 16384] buffer, folded across
   partitions at the end with PE transposes + reduces.

Numerical trick: y is kd-sorted into 32 spatially compact tiles of 512; both
point sets are translated by the tile centroid before augmentation
(d is translation invariant), and each translated coordinate is split into
bf16 hi+lo parts. All PE products are then exact and small, so the K=13
bf16 matmul reproduces d to ~f32 quality despite the catastrophic
cancellation in x2+y2-2xy (measured end-to-end rel err ~8e-5).

The host finishes with a trivial O(N) reduction: sum of row mins, min of the
8 per-core column-min vectors, mean.
"""
import sys

sys.path.insert(0, "/opt/trn_rl_repo")

import numpy as np
import ml_dtypes

import concourse.bass as bass
import concourse.tile as tile
from concourse import bacc, mybir
from concourse import bass_utils
from concourse.bass_isa import ReduceOp

BF16 = ml_dtypes.bfloat16

# Problem geometry (hardcoded per the task contract).
N = 16384          # x points
M = 16384          # y points
D = 3
NCORES = 8
XSHARD = N // NCORES        # 2048 x rows per core
P = 128                     # partitions
YTILE = 512                 # translation granularity == matmul moving width
NYT = M // YTILE            # 32 y tiles
YGRP = 2048                 # PSUM group width (4 banks)
NYG = M // YGRP             # 8 groups
NXT = XSHARD // P           # 16 x tiles per core
K = 13                      # augmented contraction depth
VPER = 3                    # XT variants packed per 128-partition page
NPAGES = (NYT + VPER - 1) // VPER   # 11
NCHUNK = M // P             # 128 column chunks of C
INF = 3.0e38
GP_PARITY = 1   # yg slices with this parity run their C-update on GPSIMD


def _bf16_pair(a):
    """Split float64 array into (hi, lo) bf16 parts."""
    hi = a.astype(BF16)
    lo = (a - hi.astype(np.float64)).astype(BF16)
    return hi, lo


def kd_sort(y, n_tiles):
    """Recursive median splits -> permutation grouping y into n_tiles
    spatially compact tiles (n_tiles must be a power of two)."""
    groups = [np.arange(len(y))]
    while len(groups) < n_tiles:
        nxt = []
        for g in groups:
            pts = y[g]
            dim = int(np.argmax(pts.max(0) - pts.min(0)))
            order = np.argsort(pts[:, dim], kind="stable")
            half = len(g) // 2
            nxt.append(g[order[:half]])
            nxt.append(g[order[half:]])
        groups = nxt
    return np.concatenate(groups)


def build_nc(n_xt=NXT, n_yg=NYG):
    """Build the SPMD Bass program (same NEFF on all cores)."""
    n_yt = n_yg * (YGRP // YTILE)
    n_pages = (n_yt + VPER - 1) // VPER
    m = n_yt * YTILE
    n_chunk = m // P
    xshard = n_xt * P
    xt_cols = n_pages * xshard
    out_w = n_xt

    nc = bacc.Bacc("TRN2", target_bir_lowering=False, debug=False,
                   num_devices=NCORES)
    xt_d = nc.dram_tensor("xt", [P, xt_cols], mybir.dt.bfloat16,
                          kind="ExternalInput")
    yt_d = nc.dram_tensor("yt", [64 + K, m], mybir.dt.bfloat16,
                          kind="ExternalInput")
    out_d = nc.dram_tensor("out", [P, out_w], mybir.dt.float32,
                           kind="ExternalOutput")
    d2_d = nc.dram_tensor("d2", [1, m], mybir.dt.float16,
                          kind="ExternalOutput")

    with tile.TileContext(nc) as tc:
        with (
            tc.tile_pool(name="const", bufs=1) as cpool,
            tc.tile_pool(name="spool", bufs=3) as spool,
            tc.tile_pool(name="apool", bufs=2) as apool,
            tc.tile_pool(name="ps", bufs=2, space="PSUM") as pspool,
        ):
            xt_t = cpool.tile([P, xt_cols], mybir.dt.bfloat16)
            yt_t = cpool.tile([64 + K, m], mybir.dt.bfloat16)
            c_t = cpool.tile([P, m], mybir.dt.float16)
            out_t = cpool.tile([P, out_w], mybir.dt.float32)
            nc.sync.dma_start(xt_t[:], xt_d.ap())
            nc.sync.dma_start(yt_t[:], yt_d.ap())

            for xt in range(n_xt):
                a_t = apool.tile([P, n_yg], mybir.dt.float32, tag="acc")
                for yg in range(n_yg):
                    ps = pspool.tile([P, YGRP], mybir.dt.float32, tag="d")
                    for j4 in range(YGRP // YTILE):
                        j = yg * (YGRP // YTILE) + j4
                        page, slot = divmod(j, VPER)
                        lhsT = xt_t[slot * 32: slot * 32 + K,
                                    page * xshard + xt * P:
                                    page * xshard + (xt + 1) * P]
                        nc.tensor.matmul(
                            ps[:, j4 * YTILE:(j4 + 1) * YTILE],
                            lhsT,
                            yt_t[slot * 32: slot * 32 + K,
                                 j * YTILE:(j + 1) * YTILE],
                            start=True, stop=True,
                        )
                    s_t = spool.tile([P, YGRP], mybir.dt.float16, tag="s")
                    cs = c_t[:, yg * YGRP:(yg + 1) * YGRP]
                    # A[:, yg] = min_j d (clean row mins); S = fp16 copy of d
                    nc.vector.tensor_reduce(
                        a_t[:, yg:yg + 1], ps[:],
                        axis=mybir.AxisListType.X, op=mybir.AluOpType.min,
                    )
                    nc.scalar.mul(s_t[:], ps[:], -1.0)
                    if xt == 0:
                        nc.vector.tensor_copy(cs, s_t[:])
                    else:
                        nc.vector.tensor_tensor(cs, s_t[:], cs,
                                                mybir.AluOpType.max)
                nc.vector.tensor_reduce(
                    out_t[:, xt:xt + 1], a_t[:],
                    axis=mybir.AxisListType.X, op=mybir.AluOpType.min,
                )

            # Fold C (holding -d maxes) across partitions in place on
            # GPSIMD, one slice at a time so the folds overlap the tail of
            # the main loop (each depends only on its slice's last update).
            for yg in range(n_yg):
                cs = c_t[:, yg * YGRP:(yg + 1) * YGRP]
                nc.gpsimd.partition_all_reduce(cs, cs, P, ReduceOp.max)
                nc.sync.dma_start(
                    d2_d.ap()[0:1, yg * YGRP:(yg + 1) * YGRP], cs[0:1, :])
            nc.sync.dma_start(out_d.ap(), out_t[:])

    nc.compile()
    return nc


def prep_inputs(x, y, n_xt=NXT, n_yg=NYG):
    """Host-side: kd-sort y, per-tile translate+augment+bf16-split, pack."""
    n_yt = n_yg * (YGRP // YTILE)
    n_pages = (n_yt + VPER - 1) // VPER
    m = n_yt * YTILE
    xshard = n_xt * P
    ncores = x.shape[0] // xshard

    perm = kd_sort(y, n_yt)
    ys = y[perm].astype(np.float64)

    yt = np.zeros((K, m), dtype=BF16)
    xts = [np.zeros((P, n_pages * xshard), dtype=BF16) for _ in range(ncores)]
    x64 = x.astype(np.float64)

    for j in range(n_yt):
        sl = slice(j * YTILE, (j + 1) * YTILE)
        yb = ys[sl]
        c = yb.mean(0)
        yp = yb - c
        yh, yl = _bf16_pair(yp)
        y2h, y2l = _bf16_pair((yp ** 2).sum(1))
        # rhs rows: yh(3), yl(3), yh(3), 1, 1, y2h, y2l
        yt[0:3, sl] = yh.T
        yt[3:6, sl] = yl.T
        yt[6:9, sl] = yh.T
        yt[9, sl] = BF16(1.0)
        yt[10, sl] = BF16(1.0)
        yt[11, sl] = y2h
        yt[12, sl] = y2l

        page, slot = divmod(j, VPER)
        xp_all = x64 - c
        x2_all = (xp_all ** 2).sum(1)
        for cidx in range(ncores):
            xp = xp_all[cidx * xshard:(cidx + 1) * xshard]
            x2 = x2_all[cidx * xshard:(cidx + 1) * xshard]
            xh, xl = _bf16_pair(xp)
            m2h = (-2.0 * xh.astype(np.float64)).astype(BF16)
            m2l = (-2.0 * xl.astype(np.float64)).astype(BF16)
            x2h, x2l = _bf16_pair(x2)
            blk = np.zeros((K, xshard), dtype=BF16)
            # lhsT rows paired with rhs rows above:
            blk[0:3] = m2h.T          # . yh
            blk[3:6] = m2h.T          # . yl
            blk[6:9] = m2l.T          # . yh
            blk[9] = x2h              # . 1
            blk[10] = x2l             # . 1
            blk[11] = BF16(1.0)       # . y2h
            blk[12] = BF16(1.0)       # . y2l
            xts[cidx][slot * 32: slot * 32 + K,
                      page * xshard:(page + 1) * xshard] = blk

    yt_rep = np.zeros((64 + K, m), dtype=BF16)
    for s in range(VPER):
        yt_rep[s * 32: s * 32 + K] = yt
    in_maps = [
        {"xt": xts[cidx], "yt": yt_rep}
        for cidx in range(ncores)
    ]
    return in_maps


def postprocess(results, n_xt=NXT, n_yg=NYG):
    n_yt = n_yg * (YGRP // YTILE)
    m = n_yt * YTILE
    d1_sum = 0.0
    d2 = np.full(m, np.inf, np.float64)
    for res in results:
        out = res["out"].astype(np.float64)
        d1_sum += np.maximum(out[:, :n_xt], 0.0).sum()
        d2 = np.minimum(d2, -res["d2"][0].astype(np.float64))
    d2_sum = np.maximum(d2, 0.0).sum()
    n_x = n_xt * P * len(results)
    return (d1_sum + d2_sum) / (n_x + m)


_NC_CACHE = {}


def kernel(x, y):
    x = np.asarray(x, np.float32)
    y = np.asarray(y, np.float32)
    key = "full"
    if key not in _NC_CACHE:
        _NC_CACHE[key] = build_nc()
    nc = _NC_CACHE[key]
    in_maps = prep_inputs(x, y)
    res = bass_utils.run_bass_kernel_spmd(nc, in_maps,
                                          core_ids=list(range(NCORES)))
    val = postprocess(res.results)
    return np.array(val, dtype=np.float32)


if __name__ == "__main__":
    np.random.seed(0)
    x = np.random.randn(N, D).astype(np.float32)
    y = np.random.randn(M, D).astype(np.float32)
    print("kernel:", kernel(x, y))


# revision 13
# speedup vs baseline: 1.0725x; 1.0322x over previous
"""Chamfer distance kernel for Trainium2 (8 NeuronCores, SPMD).

Strategy
--------
x is sharded across 8 cores (2048 rows each); y (16384 points) is replicated.
Each core computes its [2048, 16384] block of the squared-distance matrix
d_ij = |x_i - y_j|^2 via K=13 bf16 matmuls and reduces it on the fly:

 * row mins (dist1 shard) via a fused tensor_tensor_reduce per PSUM group
 * column mins (dist2 partial, min over the core's 2048 rows) via fp16
   tensor_tensor min into a resident [128, 16384] buffer, folded across
   partitions at the end with PE transposes + reduces.

Numerical trick: y is kd-sorted into 32 spatially compact tiles of 512; both
point sets are translated by the tile centroid before augmentation
(d is translation invariant), and each translated coordinate is split into
bf16 hi+lo parts. All PE products are then exact and small, so the K=13
bf16 matmul reproduces d to ~f32 quality despite the catastrophic
cancellation in x2+y2-2xy (measured end-to-end rel err ~8e-5).

The host finishes with a trivial O(N) reduction: sum of row mins, min of the
8 per-core column-min vectors, mean.
"""
import sys

sys.path.insert(0, "/opt/trn_rl_repo")

import numpy as np
import ml_dtypes

import concourse.bass as bass
import concourse.tile as tile
from concourse import bacc, mybir
from concourse import bass_utils
from concourse.bass_isa import ReduceOp

BF16 = ml_dtypes.bfloat16

# Problem geometry (hardcoded per the task contract).
N = 16384          # x points
M = 16384          # y points
D = 3
NCORES = 8
XSHARD = N // NCORES        # 2048 x rows per core
P = 128                     # partitions
YTILE = 512                 # translation granularity == matmul moving width
NYT = M // YTILE            # 32 y tiles
YGRP = 2048                 # PSUM group width (4 banks)
NYG = M // YGRP             # 8 groups
NXT = XSHARD // P           # 16 x tiles per core
K = 13                      # augmented contraction depth
VPER = 3                    # XT variants packed per 128-partition page
NPAGES = (NYT + VPER - 1) // VPER   # 11
NCHUNK = M // P             # 128 column chunks of C
INF = 3.0e38
GP_PARITY = 1   # yg slices with this parity run their C-update on GPSIMD


def _bf16_pair(a):
    """Split float64 array into (hi, lo) bf16 parts."""
    hi = a.astype(BF16)
    lo = (a - hi.astype(np.float64)).astype(BF16)
    return hi, lo


def kd_sort(y, n_tiles):
    """Recursive median splits -> permutation grouping y into n_tiles
    spatially compact tiles (n_tiles must be a power of two)."""
    groups = [np.arange(len(y))]
    while len(groups) < n_tiles:
        nxt = []
        for g in groups:
            pts = y[g]
            dim = int(np.argmax(pts.max(0) - pts.min(0)))
            order = np.argsort(pts[:, dim], kind="stable")
            half = len(g) // 2
            nxt.append(g[order[:half]])
            nxt.append(g[order[half:]])
        groups = nxt
    return np.concatenate(groups)


def build_nc(n_xt=NXT, n_yg=NYG):
    """Build the SPMD Bass program (same NEFF on all cores)."""
    n_yt = n_yg * (YGRP // YTILE)
    n_pages = (n_yt + VPER - 1) // VPER
    m = n_yt * YTILE
    n_chunk = m // P
    xshard = n_xt * P
    xt_cols = n_pages * xshard
    out_w = n_xt

    nc = bacc.Bacc("TRN2", target_bir_lowering=False, debug=False,
                   num_devices=NCORES)
    xt_d = nc.dram_tensor("xt", [P, xt_cols], mybir.dt.bfloat16,
                          kind="ExternalInput")
    yt_d = nc.dram_tensor("yt", [64 + K, m], mybir.dt.bfloat16,
                          kind="ExternalInput")
    out_d = nc.dram_tensor("out", [P, out_w], mybir.dt.float32,
                           kind="ExternalOutput")
    d2_d = nc.dram_tensor("d2", [1, m], mybir.dt.float16,
                          kind="ExternalOutput")

    with tile.TileContext(nc) as tc:
        with (
            tc.tile_pool(name="const", bufs=1) as cpool,
            tc.tile_pool(name="spool", bufs=3) as spool,
            tc.tile_pool(name="ps", bufs=2, space="PSUM") as pspool,
        ):
            xt_t = cpool.tile([P, xt_cols], mybir.dt.bfloat16)
            yt_t = cpool.tile([64 + K, m], mybir.dt.bfloat16)
            c_t = cpool.tile([P, m], mybir.dt.float16)
            out_t = cpool.tile([P, out_w], mybir.dt.float32)
            nc.sync.dma_start(xt_t[:], xt_d.ap())
            nc.sync.dma_start(yt_t[:], yt_d.ap())

            a_all = cpool.tile([P, n_xt * n_yg], mybir.dt.float32)
            # yg outer / xt inner: each C slice is final after its inner xt
            # loop, so its GPSIMD partition-fold + d2 DMA overlap the rest
            # of the main loop instead of serializing at the kernel tail.
            for yg in range(n_yg):
                cs = c_t[:, yg * YGRP:(yg + 1) * YGRP]
                for xt in range(n_xt):
                    ps = pspool.tile([P, YGRP], mybir.dt.float32, tag="d")
                    for j4 in range(YGRP // YTILE):
                        j = yg * (YGRP // YTILE) + j4
                        page, slot = divmod(j, VPER)
                        lhsT = xt_t[slot * 32: slot * 32 + K,
                                    page * xshard + xt * P:
                                    page * xshard + (xt + 1) * P]
                        nc.tensor.matmul(
                            ps[:, j4 * YTILE:(j4 + 1) * YTILE],
                            lhsT,
                            yt_t[slot * 32: slot * 32 + K,
                                 j * YTILE:(j + 1) * YTILE],
                            start=True, stop=True,
                        )
                    s_t = spool.tile([P, YGRP], mybir.dt.float16, tag="s")
                    # row-min partial of this group; S = -d in fp16
                    nc.vector.tensor_reduce(
                        a_all[:, xt * n_yg + yg: xt * n_yg + yg + 1], ps[:],
                        axis=mybir.AxisListType.X, op=mybir.AluOpType.min,
                    )
                    nc.scalar.mul(s_t[:], ps[:], -1.0)
                    if xt == 0:
                        nc.vector.tensor_copy(cs, s_t[:])
                    else:
                        nc.vector.tensor_tensor(cs, s_t[:], cs,
                                                mybir.AluOpType.max)
                nc.gpsimd.partition_all_reduce(cs, cs, P, ReduceOp.max)
                nc.sync.dma_start(
                    d2_d.ap()[0:1, yg * YGRP:(yg + 1) * YGRP], cs[0:1, :])

            for xt in range(n_xt):
                nc.vector.tensor_reduce(
                    out_t[:, xt:xt + 1],
                    a_all[:, xt * n_yg:(xt + 1) * n_yg],
                    axis=mybir.AxisListType.X, op=mybir.AluOpType.min,
                )
            nc.sync.dma_start(out_d.ap(), out_t[:])

    nc.compile()
    return nc


def prep_inputs(x, y, n_xt=NXT, n_yg=NYG):
    """Host-side: kd-sort y, per-tile translate+augment+bf16-split, pack."""
    n_yt = n_yg * (YGRP // YTILE)
    n_pages = (n_yt + VPER - 1) // VPER
    m = n_yt * YTILE
    xshard = n_xt * P
    ncores = x.shape[0] // xshard

    perm = kd_sort(y, n_yt)
    ys = y[perm].astype(np.float64)

    yt = np.zeros((K, m), dtype=BF16)
    xts = [np.zeros((P, n_pages * xshard), dtype=BF16) for _ in range(ncores)]
    x64 = x.astype(np.float64)

    for j in range(n_yt):
        sl = slice(j * YTILE, (j + 1) * YTILE)
        yb = ys[sl]
        c = yb.mean(0)
        yp = yb - c
        yh, yl = _bf16_pair(yp)
        y2h, y2l = _bf16_pair((yp ** 2).sum(1))
        # rhs rows: yh(3), yl(3), yh(3), 1, 1, y2h, y2l
        yt[0:3, sl] = yh.T
        yt[3:6, sl] = yl.T
        yt[6:9, sl] = yh.T
        yt[9, sl] = BF16(1.0)
        yt[10, sl] = BF16(1.0)
        yt[11, sl] = y2h
        yt[12, sl] = y2l

        page, slot = divmod(j, VPER)
        xp_all = x64 - c
        x2_all = (xp_all ** 2).sum(1)
        for cidx in range(ncores):
            xp = xp_all[cidx * xshard:(cidx + 1) * xshard]
            x2 = x2_all[cidx * xshard:(cidx + 1) * xshard]
            xh, xl = _bf16_pair(xp)
            m2h = (-2.0 * xh.astype(np.float64)).astype(BF16)
            m2l = (-2.0 * xl.astype(np.float64)).astype(BF16)
            x2h, x2l = _bf16_pair(x2)
            blk = np.zeros((K, xshard), dtype=BF16)
            # lhsT rows paired with rhs rows above:
            blk[0:3] = m2h.T          # . yh
            blk[3:6] = m2h.T          # . yl
            blk[6:9] = m2l.T          # . yh
            blk[9] = x2h              # . 1
            blk[10] = x2l             # . 1
            blk[11] = BF16(1.0)       # . y2h
            blk[12] = BF16(1.0)       # . y2l
            xts[cidx][slot * 32: slot * 32 + K,
                      page * xshard:(page + 1) * xshard] = blk

    yt_rep = np.zeros((64 + K, m), dtype=BF16)
    for s in range(VPER):
        yt_rep[s * 32: s * 32 + K] = yt
    in_maps = [
        {"xt": xts[cidx], "yt": yt_rep}
        for cidx in range(ncores)
    ]
    return in_maps


def postprocess(results, n_xt=NXT, n_yg=NYG):
    n_yt = n_yg * (YGRP // YTILE)
    m = n_yt * YTILE
    d1_sum = 0.0
    d2 = np.full(m, np.inf, np.float64)
    for res in results:
        out = res["out"].astype(np.float64)
        d1_sum += np.maximum(out[:, :n_xt], 0.0).sum()
        d2 = np.minimum(d2, -res["d2"][0].astype(np.float64))
    d2_sum = np.maximum(d2, 0.0).sum()
    n_x = n_xt * P * len(results)
    return (d1_sum + d2_sum) / (n_x + m)


_NC_CACHE = {}


def kernel(x, y):
    x = np.asarray(x, np.float32)
    y = np.asarray(y, np.float32)
    key = "full"
    if key not in _NC_CACHE:
        _NC_CACHE[key] = build_nc()
    nc = _NC_CACHE[key]
    in_maps = prep_inputs(x, y)
    res = bass_utils.run_bass_kernel_spmd(nc, in_maps,
                                          core_ids=list(range(NCORES)))
    val = postprocess(res.results)
    return np.array(val, dtype=np.float32)


if __name__ == "__main__":
    np.random.seed(0)
    x = np.random.randn(N, D).astype(np.float32)
    y = np.random.randn(M, D).astype(np.float32)
    print("kernel:", kernel(x, y))
